# revision 10
# baseline (speedup 1.0000x reference)
"""Trainium2 Bass kernel for nn_AtomAttention (B=2, N=2048, D=256, C=4, H=4).

Key algebraic property of the reference:

    weighted = einsum('bqkh,bvdh->bqdh', att, v)

has NO shared summation index between `att` and `v` (`k` and `v` are summed
independently), so it factorizes into

    weighted[b,q,d,h] = (sum_k att[b,q,k,h]) * (sum_v v[b,v,d,h])

and since `att` is a softmax over axis k, the first factor is exactly 1 for
every (b,q,h) — regardless of the attention scores, bias, mask or scaling.
Therefore the whole network reduces exactly (not approximately) to

    vsum[b,:]  = (sum_n atom_embed[b,n,:]) @ Wv              # (B, D*H)
    gate       = sigmoid(atom_embed @ Wg + bg)               # (B, N, D*H)
    out        = (gate * vsum[:,None,:]) @ Wo + bo           # (B, N, D)

molecular_matrix / Wq / Wk / W_bias / layernorm params / embedding_mask
cancel out of the forward value entirely, so the kernel never reads them.

Sharding: 8 cores, data-parallel over batch and sequence: core c handles
batch b=c//4, query rows [s*512,(s+1)*512); each core gets the full E[b]^T
(own 512 columns first) so the batch column-sum is local (no collectives),
plus replicated weights.

Everything runs in bf16 (fp32 PSUM/partials) — tolerance is 2e-2 and this
lands ~6e-3 — halving HBM bytes and running the PE at full bf16/FWL rate.

Scheduling notes (v5, from trace analysis of v1-v4):
- Two HWDGE rings (scalar + sync) share the ~350 B/ns HBM port at packet
  granularity. The gate path is split across BOTH rings so it completes
  first: scalar carries [bias, eo, wg(t0-3)], sync leads with wg(t4-7)
  before er/wv/wo. Gate matmuls process t=4..7 first.
- The tiny bias tensor must land FIRST: in v4 it sat behind w1 on the
  ring and its completion semaphore gated the ACT table load + the whole
  sigmoid stream until 14us.
- A dummy (128,1) sigmoid right after the DMA issues pulls the ~1.3us
  ACT_TABLE_LOAD into the transfer window instead of the sigmoid stream.
- 7 N=512 warmup matmuls span the PE-idle window [~6.9..9.9] so the HAM
  clock-gate reaches 2.4GHz as the first gate matmul issues (idle PE
  re-throttles after ~3.4us; a too-early warmup burst is wasted).
- ACT sigmoid stream (8 x ~720ns, irreducible: (N+352)/1.2 per op and
  PSUM banks cap N at 512) is the critical pipe; out matmuls run as
  interleaved (m0,m1) pairs per t in the same t-order as ACT.
- vsum -> vs_t -> wos_t pipelined per t on DVE under the ACT stream; the
  two output stores go out on different rings in parallel.
"""
import ml_dtypes
import numpy as np
import concourse.bacc as bacc
import concourse.tile as tile
from concourse import mybir
from concourse.bass_utils import run_bass_kernel_spmd

B, N, D, H = 2, 2048, 256, 4
DH = D * H
NCORES = 8
CPB = NCORES // B          # cores per batch
ROWS = N // CPB            # 512 query rows per core
REST = N - ROWS            # 1536
HREST = REST // 2          # 768 columns per er half
P = 128
KC = D // P                # 2 contraction blocks (d)
TT = DH // P               # 8 dh tiles
MC = D // P                # 2 output-d tiles
NWARM = 7
F32 = mybir.dt.float32
BF16 = mybir.dt.bfloat16
BF_NP = ml_dtypes.bfloat16

T_ORDER = [4, 5, 6, 7, 0, 1, 2, 3]  # wgB tiles (sync ring, arrives first) first


def build_nc():
    nc = bacc.Bacc("TRN2", target_bir_lowering=False, debug=False, num_devices=NCORES)
    bias = nc.dram_tensor("bias", [P, TT + MC], F32, kind="ExternalInput")
    eo = nc.dram_tensor("eo", [P, KC * ROWS], BF16, kind="ExternalInput")    # [c][512]
    wga = nc.dram_tensor("wga", [P, 4 * KC * P], BF16, kind="ExternalInput")  # t0-3 [t][c][128]
    wgb = nc.dram_tensor("wgb", [P, 4 * KC * P], BF16, kind="ExternalInput")  # t4-7 [t][c][128]
    er = nc.dram_tensor("er", [P, KC * REST], BF16, kind="ExternalInput")    # [h][c][768]
    wv = nc.dram_tensor("wv", [P, KC * DH], BF16, kind="ExternalInput")      # [c][dh]
    wo = nc.dram_tensor("wo", [P, TT * D], BF16, kind="ExternalInput")       # [t][d]
    out = nc.dram_tensor("out", [P, MC * ROWS], BF16, kind="ExternalOutput")
    with tile.TileContext(nc) as tc:
        with (
            tc.tile_pool(name="sb", bufs=1) as sb,
            tc.tile_pool(name="osb", bufs=2) as osb,
            tc.tile_pool(name="ps_w", bufs=1, space="PSUM") as ps_w,
            tc.tile_pool(name="ps_g", bufs=4, space="PSUM") as ps_g,
            tc.tile_pool(name="ps_v", bufs=1, space="PSUM") as ps_v,
            tc.tile_pool(name="ps_o", bufs=1, space="PSUM") as ps_o,
        ):
            warm = sb.tile([P, ROWS], BF16, tag="warm")
            dmy = sb.tile([P, 1], BF16, tag="dmy")
            bias_sb = sb.tile([P, TT + MC], F32, tag="bias")
            eo_sb = sb.tile([P, KC * ROWS], BF16, tag="eo")
            wga_sb = sb.tile([P, 4 * KC * P], BF16, tag="wga")
            wgb_sb = sb.tile([P, 4 * KC * P], BF16, tag="wgb")
            er_sb = sb.tile([P, KC * REST], BF16, tag="er")
            wv_sb = sb.tile([P, KC * DH], BF16, tag="wv")
            wo_sb = sb.tile([P, TT * D], BF16, tag="wo")
            # esum partials: [c: own, h0, h1]
            esp = sb.tile([P, KC, 3], F32, tag="esp")
            esp_bf = sb.tile([P, KC, 3], BF16, tag="espb")
            gt = [sb.tile([P, ROWS], BF16, name=f"gt{t}", tag=f"gt{t}")
                  for t in range(TT)]
            vs_f = sb.tile([P, TT], F32, tag="vsf")
            wos = [sb.tile([P, D], BF16, name=f"wos{t}", tag=f"wos{t}") for t in range(TT)]

            def wg_ap(c, t):
                w_sb = wga_sb if t < 4 else wgb_sb
                return w_sb[:, ((t % 4) * KC + c) * P:((t % 4) * KC + c) * P + P]

            def eo_ap(c):
                return eo_sb[:, c * ROWS:(c + 1) * ROWS]

            # --- PE warmup: span the DMA wait so HAM hits 2.4GHz at first gate MM
            nc.vector.memset(warm[:], 0.0)
            wps = ps_w.tile([P, ROWS], F32)
            for _ in range(NWARM):
                nc.tensor.matmul(wps[:], warm[:, 0:P], warm[:], start=True, stop=True)

            # --- input DMAs (per-ring FIFO): gate path split across both rings
            nc.scalar.dma_start(bias_sb[:], bias[:])
            nc.scalar.dma_start(eo_sb[:], eo[:])
            nc.scalar.dma_start(wga_sb[:], wga[:])
            nc.sync.dma_start(wgb_sb[:], wgb[:])
            for h in range(2):
                w = KC * HREST
                nc.sync.dma_start(er_sb[:, h * w:(h + 1) * w], er[:, h * w:(h + 1) * w])
            nc.sync.dma_start(wv_sb[:], wv[:])
            nc.sync.dma_start(wo_sb[:], wo[:])

            # --- dummy sigmoid: hoist the ACT table load into the DMA window
            nc.scalar.activation(dmy[:], warm[:, 0:1],
                                 mybir.ActivationFunctionType.Sigmoid)

            # --- gate: gT_t = sigmoid(Wg_t^T @ E_own^T + bg_t), bf16
            for t in T_ORDER:
                g_ps = ps_g.tile([P, ROWS], F32)
                for c in range(KC):
                    nc.tensor.matmul(g_ps[:], wg_ap(c, t), eo_ap(c),
                                     start=(c == 0), stop=(c == KC - 1))
                nc.scalar.activation(gt[t][:], g_ps[:],
                                     mybir.ActivationFunctionType.Sigmoid,
                                     bias=bias_sb[:, t:t + 1])

            # --- esum partials (fp32) on DVE as data lands
            for c in range(KC):
                nc.vector.reduce_sum(esp[:, c, 0:1], eo_ap(c), axis=mybir.AxisListType.X)
            for h in range(2):
                for c in range(KC):
                    base = h * KC * HREST + c * HREST
                    nc.vector.reduce_sum(esp[:, c, 1 + h:2 + h],
                                         er_sb[:, base:base + HREST],
                                         axis=mybir.AxisListType.X)
            nc.vector.tensor_copy(esp_bf[:], esp[:])

            # --- vsum -> vs_t -> wos_t pipelined per t
            vs_ps = ps_v.tile([P, TT, 3], F32)
            for t in T_ORDER:
                for c in range(KC):
                    nc.tensor.matmul(vs_ps[:, t, :],
                                     wv_sb[:, c * DH + t * P:c * DH + (t + 1) * P],
                                     esp_bf[:, c, :],
                                     start=(c == 0), stop=(c == KC - 1))
                nc.vector.reduce_sum(vs_f[:, t:t + 1], vs_ps[:, t, :],
                                     axis=mybir.AxisListType.X)
                nc.vector.tensor_scalar_mul(wos[t][:], wo_sb[:, t * D:(t + 1) * D],
                                            vs_f[:, t:t + 1])

            # --- out: outT_m = sum_t wos_t[:,m]^T @ gT_t (+ bo); m groups
            # interleaved per t (same t-order as the ACT stream)
            o_ps = [ps_o.tile([P, ROWS], F32, name=f"ops{m}") for m in range(MC)]
            for i, t in enumerate(T_ORDER):
                for m in range(MC):
                    nc.tensor.matmul(o_ps[m][:], wos[t][:, m * P:(m + 1) * P],
                                     gt[t][:], start=(i == 0), stop=(i == TT - 1))
            for m in range(MC):
                o_sb = osb.tile([P, ROWS], BF16, name="o", tag="o")
                nc.vector.tensor_scalar_add(o_sb[:], o_ps[m][:],
                                            bias_sb[:, TT + m:TT + m + 1])
                eng = nc.scalar if m == 0 else nc.sync
                eng.dma_start(out[:, m * ROWS:(m + 1) * ROWS], o_sb[:])
    nc.compile()
    return nc


_NC = None


def _get_nc():
    global _NC
    if _NC is None:
        _NC = build_nc()
    return _NC


def _make_in_maps(inputs):
    E = np.asarray(inputs["atom_embed"], dtype=np.float32)
    Wg = np.asarray(inputs["Wg"], dtype=np.float32)
    Wv = np.asarray(inputs["Wv"], dtype=np.float32)
    Wo = np.asarray(inputs["Wo"], dtype=np.float32)
    bg = np.asarray(inputs["bg"], dtype=np.float32)
    bo = np.asarray(inputs["bo"], dtype=np.float32)

    # wg in t-major [t][c][128] layout, split in t-halves across the rings
    wg_t = np.concatenate(
        [Wg[c * P:(c + 1) * P, t * P:(t + 1) * P] for t in range(TT) for c in range(KC)],
        axis=1).astype(BF_NP)  # (128, 2048)
    wga_np = np.ascontiguousarray(wg_t[:, :4 * KC * P])
    wgb_np = np.ascontiguousarray(wg_t[:, 4 * KC * P:])
    wv_np = np.ascontiguousarray(np.concatenate(
        [Wv[c * P:(c + 1) * P, :] for c in range(KC)], axis=1)).astype(BF_NP)
    wo_np = np.ascontiguousarray(np.concatenate(
        [Wo[t * P:(t + 1) * P, :] for t in range(TT)], axis=1)).astype(BF_NP)
    bias_np = np.ascontiguousarray(np.concatenate(
        [bg.reshape(TT, P).T, bo.reshape(MC, P).T], axis=1))  # (128, 10) f32

    in_maps = []
    for core in range(NCORES):
        b, s = divmod(core, CPB)
        ET = E[b].T.astype(BF_NP)  # (D, N) bf16
        own = ET[:, s * ROWS:(s + 1) * ROWS]
        rest = np.concatenate([ET[:, (s + 1) * ROWS:], ET[:, :s * ROWS]], axis=1)
        eo_np = np.concatenate([own[c * P:(c + 1) * P, :] for c in range(KC)], axis=1)
        er_np = np.concatenate(
            [rest[c * P:(c + 1) * P, h * HREST:(h + 1) * HREST]
             for h in range(2) for c in range(KC)], axis=1)
        in_maps.append({
            "bias": bias_np, "eo": np.ascontiguousarray(eo_np),
            "wga": wga_np, "wgb": wgb_np,
            "er": np.ascontiguousarray(er_np),
            "wv": wv_np, "wo": wo_np,
        })
    return in_maps


def _run(inputs, trace=False):
    """Run on 8 NeuronCores; returns (full_output, BassKernelResults)."""
    in_maps = _make_in_maps(inputs)
    res = run_bass_kernel_spmd(_get_nc(), in_maps, list(range(NCORES)),
                               trace=trace)
    out = np.empty((B, N, D), dtype=np.float32)
    for core in range(NCORES):
        b, s = divmod(core, CPB)
        o = res.results[core]["out"]  # (128, 2*512) bf16, m-major
        oT = np.concatenate([o[:, m * ROWS:(m + 1) * ROWS] for m in range(MC)],
                            axis=0).astype(np.float32)  # (256, 512)
        out[b, s * ROWS:(s + 1) * ROWS, :] = oT.T
    return out, res


def kernel(**inputs) -> np.ndarray:
    out, _ = _run(inputs, trace=False)
    return out


# revision 11
# speedup vs baseline: 1.0489x; 1.0489x over previous
"""Trainium2 Bass kernel for nn_AtomAttention (B=2, N=2048, D=256, C=4, H=4).

Key algebraic property of the reference:

    weighted = einsum('bqkh,bvdh->bqdh', att, v)

has NO shared summation index between `att` and `v` (`k` and `v` are summed
independently), so it factorizes into

    weighted[b,q,d,h] = (sum_k att[b,q,k,h]) * (sum_v v[b,v,d,h])

and since `att` is a softmax over axis k, the first factor is exactly 1 for
every (b,q,h) — regardless of the attention scores, bias, mask or scaling.
Therefore the whole network reduces exactly (not approximately) to

    vsum[b,:]  = (sum_n atom_embed[b,n,:]) @ Wv              # (B, D*H)
    gate       = sigmoid(atom_embed @ Wg + bg)               # (B, N, D*H)
    out        = (gate * vsum[:,None,:]) @ Wo + bo           # (B, N, D)

molecular_matrix / Wq / Wk / W_bias / layernorm params / embedding_mask
cancel out of the forward value entirely, so the kernel never reads them.

Sharding: 8 cores, data-parallel over batch and sequence: core c handles
batch b=c//4, query rows [s*512,(s+1)*512); each core gets the full E[b]^T
(own 512 columns first) so the batch column-sum is local (no collectives),
plus replicated weights.

Everything runs in bf16 (fp32 PSUM/partials) — tolerance is 2e-2 and this
lands ~6e-3 — halving HBM bytes and running the PE at full bf16/FWL rate.

Scheduling notes (v6, from trace analysis of v1-v5):
- DMA bandwidth is shared per-QUEUE across the two HWDGE rings: a lone
  transfer on the scalar ring gets ~1/8 of the port while the sync ring
  has 5 queued (v5: a 256KB eo took 5.8us and stalled everything). All
  input transfers therefore ride the SYNC ring as one strict FIFO in
  consumption order: [bias, eo, wg(t0-3), wg(t4-7), er h0, er h1, wv, wo].
  No data DMA can start before ~8.2us (runtime iram gating) on any ring.
- A dummy (128,1) sigmoid on the scalar engine preloads the ~2.6us of
  ACT_TABLE_LOADs during the DMA window instead of the sigmoid stream.
- 7 N=512 warmup matmuls span the PE-idle window [~6.9..9.9] so the HAM
  clock-gate reaches 2.4GHz as the first gate matmul issues (idle PE
  re-throttles after ~3.4us; a too-early warmup burst is wasted).
- ACT sigmoid stream (8 x ~720ns, irreducible: (N+352)/1.2 per op and
  PSUM banks cap N at 512) is the critical pipe; out matmuls run as
  interleaved (m0,m1) pairs per t in the same t-order as ACT.
- vsum -> vs_t -> wos_t pipelined per t on DVE under the ACT stream; the
  two output stores go out on different rings in parallel.
"""
import ml_dtypes
import numpy as np
import concourse.bacc as bacc
import concourse.tile as tile
from concourse import mybir
from concourse.bass_utils import run_bass_kernel_spmd

B, N, D, H = 2, 2048, 256, 4
DH = D * H
NCORES = 8
CPB = NCORES // B          # cores per batch
ROWS = N // CPB            # 512 query rows per core
REST = N - ROWS            # 1536
HREST = REST // 2          # 768 columns per er half
P = 128
KC = D // P                # 2 contraction blocks (d)
TT = DH // P               # 8 dh tiles
MC = D // P                # 2 output-d tiles
NWARM = 7
F32 = mybir.dt.float32
BF16 = mybir.dt.bfloat16
BF_NP = ml_dtypes.bfloat16

T_ORDER = list(range(TT))  # wgA (t0-3) arrives first on the FIFO


def build_nc():
    nc = bacc.Bacc("TRN2", target_bir_lowering=False, debug=False, num_devices=NCORES)
    bias = nc.dram_tensor("bias", [P, TT + MC], F32, kind="ExternalInput")
    eo = nc.dram_tensor("eo", [P, KC * ROWS], BF16, kind="ExternalInput")    # [c][512]
    wga = nc.dram_tensor("wga", [P, 4 * KC * P], BF16, kind="ExternalInput")  # t0-3 [t][c][128]
    wgb = nc.dram_tensor("wgb", [P, 4 * KC * P], BF16, kind="ExternalInput")  # t4-7 [t][c][128]
    er = nc.dram_tensor("er", [P, KC * REST], BF16, kind="ExternalInput")    # [h][c][768]
    wv = nc.dram_tensor("wv", [P, KC * DH], BF16, kind="ExternalInput")      # [c][dh]
    wo = nc.dram_tensor("wo", [P, TT * D], BF16, kind="ExternalInput")       # [t][d]
    out = nc.dram_tensor("out", [P, MC * ROWS], BF16, kind="ExternalOutput")
    with tile.TileContext(nc) as tc:
        with (
            tc.tile_pool(name="sb", bufs=1) as sb,
            tc.tile_pool(name="osb", bufs=2) as osb,
            tc.tile_pool(name="ps_w", bufs=1, space="PSUM") as ps_w,
            tc.tile_pool(name="ps_g", bufs=4, space="PSUM") as ps_g,
            tc.tile_pool(name="ps_v", bufs=1, space="PSUM") as ps_v,
            tc.tile_pool(name="ps_o", bufs=1, space="PSUM") as ps_o,
        ):
            warm = sb.tile([P, ROWS], BF16, tag="warm")
            dmy = sb.tile([P, 1], BF16, tag="dmy")
            bias_sb = sb.tile([P, TT + MC], F32, tag="bias")
            eo_sb = sb.tile([P, KC * ROWS], BF16, tag="eo")
            wga_sb = sb.tile([P, 4 * KC * P], BF16, tag="wga")
            wgb_sb = sb.tile([P, 4 * KC * P], BF16, tag="wgb")
            er_sb = sb.tile([P, KC * REST], BF16, tag="er")
            wv_sb = sb.tile([P, KC * DH], BF16, tag="wv")
            wo_sb = sb.tile([P, TT * D], BF16, tag="wo")
            # esum partials: [c: own, h0, h1]
            esp = sb.tile([P, KC, 3], F32, tag="esp")
            esp_bf = sb.tile([P, KC, 3], BF16, tag="espb")
            gt = [sb.tile([P, ROWS], BF16, name=f"gt{t}", tag=f"gt{t}")
                  for t in range(TT)]
            vs_f = sb.tile([P, TT], F32, tag="vsf")
            wos = [sb.tile([P, D], BF16, name=f"wos{t}", tag=f"wos{t}") for t in range(TT)]

            def wg_ap(c, t):
                w_sb = wga_sb if t < 4 else wgb_sb
                return w_sb[:, ((t % 4) * KC + c) * P:((t % 4) * KC + c) * P + P]

            def eo_ap(c):
                return eo_sb[:, c * ROWS:(c + 1) * ROWS]

            # --- PE warmup: span the DMA wait so HAM hits 2.4GHz at first gate MM
            nc.vector.memset(warm[:], 0.0)
            wps = ps_w.tile([P, ROWS], F32)
            for _ in range(NWARM):
                nc.tensor.matmul(wps[:], warm[:, 0:P], warm[:], start=True, stop=True)

            # --- input DMAs: ONE strict FIFO on the sync ring, consumption order
            nc.sync.dma_start(bias_sb[:], bias[:])
            nc.sync.dma_start(eo_sb[:], eo[:])
            nc.sync.dma_start(wga_sb[:], wga[:])
            nc.sync.dma_start(wgb_sb[:], wgb[:])
            for h in range(2):
                w = KC * HREST
                nc.sync.dma_start(er_sb[:, h * w:(h + 1) * w], er[:, h * w:(h + 1) * w])
            nc.sync.dma_start(wv_sb[:], wv[:])
            nc.sync.dma_start(wo_sb[:], wo[:])

            # --- dummy sigmoid: hoist the ACT table load into the DMA window
            nc.scalar.activation(dmy[:], warm[:, 0:1],
                                 mybir.ActivationFunctionType.Sigmoid)

            # --- gate: gT_t = sigmoid(Wg_t^T @ E_own^T + bg_t), bf16
            for t in T_ORDER:
                g_ps = ps_g.tile([P, ROWS], F32)
                for c in range(KC):
                    nc.tensor.matmul(g_ps[:], wg_ap(c, t), eo_ap(c),
                                     start=(c == 0), stop=(c == KC - 1))
                nc.scalar.activation(gt[t][:], g_ps[:],
                                     mybir.ActivationFunctionType.Sigmoid,
                                     bias=bias_sb[:, t:t + 1])

            # --- esum partials (fp32) on DVE as data lands
            for c in range(KC):
                nc.vector.reduce_sum(esp[:, c, 0:1], eo_ap(c), axis=mybir.AxisListType.X)
            for h in range(2):
                for c in range(KC):
                    base = h * KC * HREST + c * HREST
                    nc.vector.reduce_sum(esp[:, c, 1 + h:2 + h],
                                         er_sb[:, base:base + HREST],
                                         axis=mybir.AxisListType.X)
            nc.vector.tensor_copy(esp_bf[:], esp[:])

            # --- vsum -> vs_t -> wos_t pipelined per t
            vs_ps = ps_v.tile([P, TT, 3], F32)
            for t in T_ORDER:
                for c in range(KC):
                    nc.tensor.matmul(vs_ps[:, t, :],
                                     wv_sb[:, c * DH + t * P:c * DH + (t + 1) * P],
                                     esp_bf[:, c, :],
                                     start=(c == 0), stop=(c == KC - 1))
                nc.vector.reduce_sum(vs_f[:, t:t + 1], vs_ps[:, t, :],
                                     axis=mybir.AxisListType.X)
                nc.vector.tensor_scalar_mul(wos[t][:], wo_sb[:, t * D:(t + 1) * D],
                                            vs_f[:, t:t + 1])

            # --- out: outT_m = sum_t wos_t[:,m]^T @ gT_t (+ bo); m groups
            # interleaved per t (same t-order as the ACT stream)
            o_ps = [ps_o.tile([P, ROWS], F32, name=f"ops{m}") for m in range(MC)]
            for i, t in enumerate(T_ORDER):
                for m in range(MC):
                    nc.tensor.matmul(o_ps[m][:], wos[t][:, m * P:(m + 1) * P],
                                     gt[t][:], start=(i == 0), stop=(i == TT - 1))
            for m in range(MC):
                o_sb = osb.tile([P, ROWS], BF16, name="o", tag="o")
                nc.vector.tensor_scalar_add(o_sb[:], o_ps[m][:],
                                            bias_sb[:, TT + m:TT + m + 1])
                eng = nc.scalar if m == 0 else nc.sync
                eng.dma_start(out[:, m * ROWS:(m + 1) * ROWS], o_sb[:])
    nc.compile()
    return nc


_NC = None


def _get_nc():
    global _NC
    if _NC is None:
        _NC = build_nc()
    return _NC


def _make_in_maps(inputs):
    E = np.asarray(inputs["atom_embed"], dtype=np.float32)
    Wg = np.asarray(inputs["Wg"], dtype=np.float32)
    Wv = np.asarray(inputs["Wv"], dtype=np.float32)
    Wo = np.asarray(inputs["Wo"], dtype=np.float32)
    bg = np.asarray(inputs["bg"], dtype=np.float32)
    bo = np.asarray(inputs["bo"], dtype=np.float32)

    # wg in t-major [t][c][128] layout, split in t-halves across the rings
    wg_t = np.concatenate(
        [Wg[c * P:(c + 1) * P, t * P:(t + 1) * P] for t in range(TT) for c in range(KC)],
        axis=1).astype(BF_NP)  # (128, 2048)
    wga_np = np.ascontiguousarray(wg_t[:, :4 * KC * P])
    wgb_np = np.ascontiguousarray(wg_t[:, 4 * KC * P:])
    wv_np = np.ascontiguousarray(np.concatenate(
        [Wv[c * P:(c + 1) * P, :] for c in range(KC)], axis=1)).astype(BF_NP)
    wo_np = np.ascontiguousarray(np.concatenate(
        [Wo[t * P:(t + 1) * P, :] for t in range(TT)], axis=1)).astype(BF_NP)
    bias_np = np.ascontiguousarray(np.concatenate(
        [bg.reshape(TT, P).T, bo.reshape(MC, P).T], axis=1))  # (128, 10) f32

    in_maps = []
    for core in range(NCORES):
        b, s = divmod(core, CPB)
        ET = E[b].T.astype(BF_NP)  # (D, N) bf16
        own = ET[:, s * ROWS:(s + 1) * ROWS]
        rest = np.concatenate([ET[:, (s + 1) * ROWS:], ET[:, :s * ROWS]], axis=1)
        eo_np = np.concatenate([own[c * P:(c + 1) * P, :] for c in range(KC)], axis=1)
        er_np = np.concatenate(
            [rest[c * P:(c + 1) * P, h * HREST:(h + 1) * HREST]
             for h in range(2) for c in range(KC)], axis=1)
        in_maps.append({
            "bias": bias_np, "eo": np.ascontiguousarray(eo_np),
            "wga": wga_np, "wgb": wgb_np,
            "er": np.ascontiguousarray(er_np),
            "wv": wv_np, "wo": wo_np,
        })
    return in_maps


def _run(inputs, trace=False):
    """Run on 8 NeuronCores; returns (full_output, BassKernelResults)."""
    in_maps = _make_in_maps(inputs)
    res = run_bass_kernel_spmd(_get_nc(), in_maps, list(range(NCORES)),
                               trace=trace)
    out = np.empty((B, N, D), dtype=np.float32)
    for core in range(NCORES):
        b, s = divmod(core, CPB)
        o = res.results[core]["out"]  # (128, 2*512) bf16, m-major
        oT = np.concatenate([o[:, m * ROWS:(m + 1) * ROWS] for m in range(MC)],
                            axis=0).astype(np.float32)  # (256, 512)
        out[b, s * ROWS:(s + 1) * ROWS, :] = oT.T
    return out, res


def kernel(**inputs) -> np.ndarray:
    out, _ = _run(inputs, trace=False)
    return out
